# revision 2
# baseline (speedup 1.0000x reference)
"""NT-Xent / SimCLR contrastive loss on 8 Trainium2 NeuronCores.

Strategy (data-parallel over rows, fp8 DoubleRow matmuls):
  - Host: reps = concat(z_i, z_j) -> [8192, 512] fp32. Core i receives
    reps rolled by -1024*i rows so that *its* 1024 rows sit at rows 0..1023.
    SPMD program identical on every core; positive pairs at col = row+4096.
  - Device (per core):
      phase A (per 2048-row group): load rows fp32 (Act hw queue),
        square+rowsum (DVE), inv = 16/||row|| via ln/exp (ACT, one table
        set), scale rows to bf16*16 (DVE), bounce through DRAM and
        dma-transpose (Sync hw queue) into repsT bf16 [512, 8192-perm],
        then DVE-cast to fp8e4 DoubleRow pair tiles [128, 2, cols].
      phase B (interleaved after each group): psum[128,2048] = 256*sim
        slice via fp8 DoubleRow matmuls (K=256 per call, 2 calls);
        ACT exp(2*sim) with fused row-sum; DVE extracts self/positive
        diagonals via identity mask.
      epilogue: denom = rowsum - exp(2*sim_self); partial row loss is
        ln(denom) - 2*pos; partition-sum via ones-matmul; DMA scalar out.
  - Host: loss = sum(core partials) / 8192.
"""

import sys
import threading
from unittest import mock

sys.path.insert(0, "/opt/trn_rl_repo")

import numpy as np  # noqa: E402

import concourse.tile as tile  # noqa: E402
from concourse import bacc, mybir  # noqa: E402
from concourse.bass_utils import run_bass_kernel_spmd  # noqa: E402
from concourse.hw_specs import get_activation_tables  # noqa: E402
from concourse.masks import make_identity  # noqa: E402
from contextlib import ExitStack  # noqa: E402

P = 128
D = 512
TWO_N = 8192
N_CORES = 8
ROWS_PER_CORE = TWO_N // N_CORES  # 1024
T_INV = 2.0  # 1 / temperature (0.5)
S = 16.0  # fp8 pre-scale on normalized rows; psum = S*S*sim
SS = S * S

KC = D // P  # 4 contraction chunks of 128
KK = 2  # DoubleRow pairs (K=256 each)
NB = 4  # big column blocks (= row groups in phase A)
CB = TWO_N // NB  # 2048 columns per block / rows per group
TPG = CB // P  # 16 [128, 512] row tiles per group
MB = ROWS_PER_CORE // P  # 8 m-blocks of 128 rows per core
NS = CB // 512  # 4 matmul sub-columns of 512 per block

FP32 = mybir.dt.float32
BF16 = mybir.dt.bfloat16
FP8 = mybir.dt.float8e4
AF = mybir.ActivationFunctionType
ALU = mybir.AluOpType
AX = mybir.AxisListType
DR = mybir.MatmulPerfMode.DoubleRow


def _filtered_activation_tables(arch):
    """Steer every Exp/Ln/Copy activation to the one table set containing
    both Exp and Ln, so the table-load pass cannot thrash between sets."""
    tables = get_activation_tables(arch)
    target = None
    for name, funcs in tables.items():
        if AF.Exp in funcs and AF.Ln in funcs:
            target = name
            break
    if target is None:
        return tables
    steer = {AF.Exp, AF.Ln, AF.Copy, AF.Identity}
    return {
        name: (funcs if name == target else funcs - steer)
        for name, funcs in tables.items()
    }


def _build_kernel():
    nc = bacc.Bacc("TRN2", target_bir_lowering=False, debug=False,
                   num_devices=N_CORES)
    reps = nc.dram_tensor("reps", [TWO_N, D], FP32, kind="ExternalInput").ap()
    out = nc.dram_tensor("out", [1, 1], FP32, kind="ExternalOutput").ap()

    with tile.TileContext(nc) as tc, ExitStack() as ctx:
        rows_pool = ctx.enter_context(tc.tile_pool(name="rows", bufs=2))
        normed_pool = ctx.enter_context(tc.tile_pool(name="normed", bufs=2))
        sq_pool = ctx.enter_context(tc.tile_pool(name="sq", bufs=2))
        stats_pool = ctx.enter_context(tc.tile_pool(name="stats", bufs=1))
        repsTb_pool = ctx.enter_context(tc.tile_pool(name="repsTb", bufs=2))
        repsT_pool = ctx.enter_context(tc.tile_pool(name="repsT", bufs=1))
        dram_pool = ctx.enter_context(
            tc.tile_pool(name="scratch", bufs=KC * 2, space="DRAM"))
        psum_pool = ctx.enter_context(
            tc.tile_pool(name="psum", bufs=2, space="PSUM"))
        exp_pool = ctx.enter_context(tc.tile_pool(name="exp", bufs=2))
        junk_pool = ctx.enter_context(tc.tile_pool(name="junk", bufs=2))
        epi_pool = ctx.enter_context(tc.tile_pool(name="epi", bufs=1))

        # --- constants -----------------------------------------------------
        ident = stats_pool.tile([P, P], FP32, tag="ident", name="ident")
        make_identity(nc, ident[:])
        ones = stats_pool.tile([P, 1], FP32, tag="ones", name="ones")
        nc.gpsimd.memset(ones[:], 1.0)

        # accumulators for the main loop
        rs_all = stats_pool.tile([P, MB * NB], FP32, tag="rs", name="rs_all")
        e_self = stats_pool.tile([P, MB], FP32, tag="eself", name="e_self")
        pos = stats_pool.tile([P, MB], FP32, tag="pos", name="pos")

        # fp8 DoubleRow tiles: rT8[g][kk] [128, 2*2048]; [p, i*CB + q] =
        # repsT row (kk*256 + i*128 + p), permuted group column q.
        rT8 = [[repsT_pool.tile([P, KK * CB], FP8, tag=f"rT8_{g}_{kk}",
                                name=f"rT8_{g}_{kk}")
                for kk in range(KK)]
               for g in range(NB)]
        # own 1024 columns, un-permuted, contiguous for LDWEIGHTS:
        # rT0[kk] [128, 2*1024]
        rT0 = [repsT_pool.tile([P, KK * ROWS_PER_CORE], FP8, tag=f"rT0_{kk}",
                               name=f"rT0_{kk}")
               for kk in range(KK)]

        def phase_a(g):
            rows_g = rows_pool.tile([P, TPG * D], FP32, tag="rows",
                                    name=f"rows_{g}")
            src = reps[g * CB:(g + 1) * CB, :].rearrange(
                "(t p) d -> p t d", p=P)
            # input loads ride the Act hw queue; everything else on Sync
            nc.scalar.dma_start(
                out=rows_g[:].rearrange("p (t d) -> p t d", d=D), in_=src)

            n2 = stats_pool.tile([P, TPG], FP32, tag="n2", bufs=2,
                                 name=f"n2_{g}")
            for t in range(TPG):
                sq = sq_pool.tile([P, D], BF16, tag="sq", name=f"sq_{g}_{t}")
                rt = rows_g[:, t * D:(t + 1) * D]
                nc.vector.scalar_tensor_tensor(
                    out=sq[:], in0=rt, scalar=1.0, in1=rt,
                    op0=ALU.mult, op1=ALU.mult, accum_out=n2[:, t:t + 1])
            # inv = S * n2**-0.5 = exp(-0.5 * ln(n2/S^2))
            lnn = stats_pool.tile([P, TPG], FP32, tag="lnn", bufs=2,
                                  name=f"lnn_{g}")
            nc.scalar.activation(lnn[:], n2[:], AF.Ln, scale=1.0 / SS)
            inv = stats_pool.tile([P, TPG], FP32, tag="inv", bufs=2,
                                  name=f"inv_{g}")
            nc.scalar.activation(inv[:], lnn[:], AF.Exp, scale=-0.5)

            normed_g = normed_pool.tile([P, TPG * D], BF16, tag="normed",
                                        name=f"normed_{g}")
            for t in range(TPG):
                src_t = rows_g[:, t * D:(t + 1) * D]
                dst_t = normed_g[:, t * D:(t + 1) * D]
                if t % 2 == 0:
                    nc.vector.tensor_scalar_mul(dst_t, src_t, inv[:, t:t + 1])
                else:
                    nc.gpsimd.tensor_scalar_mul(dst_t, src_t, inv[:, t:t + 1])
            # Bounce through DRAM per d-chunk, permuted so both the store
            # (4 KiB runs per partition) and the transpose read (fully
            # contiguous) are DMA-friendly. Scratch row q = p*16 + t holds
            # normalized row t*128 + p.
            nview = normed_g[:].rearrange("p (t e) -> p t e", e=D)
            for k in range(KC):
                scr = dram_pool.tile([CB, P], BF16, tag=f"scr{k}",
                                     name=f"scr_{k}_{g}")
                nc.sync.dma_start(
                    out=scr[:].rearrange("(p t) c -> p t c", p=P),
                    in_=nview[:, :, k * P:(k + 1) * P])
                rTb = repsTb_pool.tile([P, CB], BF16, tag=f"rTb{k}",
                                       name=f"rTb_{k}_{g}")
                nc.sync.dma_start_transpose(rTb[:], scr[:])
                # cast bf16 -> fp8e4 into the DoubleRow pair slot
                kk, i = k // 2, k % 2
                dst = rT8[g][kk][:, i * CB:(i + 1) * CB]
                if k % 2 == 0:
                    nc.vector.tensor_copy(dst, rTb[:])
                else:
                    nc.gpsimd.tensor_copy(dst, rTb[:])
                if g == 0:
                    # un-permute own 1024 columns for contiguous lhsT:
                    # rT0 col (m*128+j) = rTb col (16j+m)
                    dst0 = rT0[kk][:, i * ROWS_PER_CORE:
                                   (i + 1) * ROWS_PER_CORE]
                    nc.vector.tensor_copy(
                        dst0.rearrange("p (m j) -> p m j", j=P),
                        rTb[:].rearrange(
                            "p (j m) -> p m j", m=TPG)[:, :MB, :])

        # permuted-column selector: columns for m-block rows m*128..m*128+127
        # sit at positions 16*j + m (j = psum row).
        def colsel(ap_2d, m):
            return ap_2d.rearrange("p (j s) -> p s j", s=TPG)[:, m, :]

        def phase_b(nb):
            for m in range(MB):
                ps = psum_pool.tile([P, CB], FP32, tag="ps",
                                    name=f"ps_{nb}_{m}")
                for kk in range(KK):
                    lhsT = rT0[kk][:].rearrange(
                        "p (i c) -> p i c", i=KK)[:, :, m * P:(m + 1) * P]
                    rview = rT8[nb][kk][:].rearrange(
                        "p (i c) -> p i c", i=KK)
                    for ns in range(NS):
                        nc.tensor.matmul(
                            ps[:, ns * 512:(ns + 1) * 512],
                            lhsT=lhsT,
                            rhs=rview[:, :, ns * 512:(ns + 1) * 512],
                            start=(kk == 0), stop=(kk == KK - 1),
                            perf_mode=DR)
                et = exp_pool.tile([P, CB], BF16, tag="et", name=f"et_{nb}_{m}")
                nc.scalar.activation(
                    et[:], ps[:], AF.Exp, scale=T_INV / SS,
                    accum_out=rs_all[:, m * NB + nb:m * NB + nb + 1])
                if nb == 0:
                    # self-similarity column: global col = row = m*128 + j,
                    # at permuted position 16*j + m.
                    junk = junk_pool.tile([P, P], FP32, tag="junk",
                                          name=f"junk_s_{m}")
                    nc.vector.scalar_tensor_tensor(
                        out=junk[:], in0=colsel(et[:], m),
                        scalar=1.0, in1=ident[:],
                        op0=ALU.mult, op1=ALU.mult,
                        accum_out=e_self[:, m:m + 1])
                if nb == 2:
                    # positive column: global col = 4096 + row, in-group
                    # offset = row -> same permuted position 16*j + m.
                    junk = junk_pool.tile([P, P], FP32, tag="junk",
                                          name=f"junk_p_{m}")
                    nc.vector.scalar_tensor_tensor(
                        out=junk[:], in0=colsel(ps[:], m),
                        scalar=1.0, in1=ident[:],
                        op0=ALU.mult, op1=ALU.mult,
                        accum_out=pos[:, m:m + 1])

        # interleave: phase B for group g right after its phase A, so the
        # tile scheduler can overlap group g+1's loads with group g's mms.
        for g in range(NB):
            phase_a(g)
            phase_b(g)

        # --- epilogue ------------------------------------------------------
        sums = epi_pool.tile([P, MB], FP32, tag="sums", name="sums")
        nc.vector.tensor_reduce(
            sums[:], rs_all[:].rearrange("p (m b) -> p m b", b=NB),
            axis=AX.X, op=ALU.add)
        denom = epi_pool.tile([P, MB], FP32, tag="denom", name="denom")
        nc.vector.tensor_sub(denom[:], sums[:], e_self[:])
        ld = epi_pool.tile([P, MB], FP32, tag="ld", name="ld")
        nc.scalar.activation(ld[:], denom[:], AF.Ln)
        # partial = ld - (2/SS)*pos   (pos holds 256*sim)
        part = epi_pool.tile([P, MB], FP32, tag="part", name="part")
        nc.vector.scalar_tensor_tensor(
            out=part[:], in0=pos[:], scalar=-T_INV / SS, in1=ld[:],
            op0=ALU.mult, op1=ALU.add)
        rowtot = epi_pool.tile([P, 1], FP32, tag="rowtot", name="rowtot")
        nc.vector.tensor_reduce(rowtot[:], part[:], axis=AX.X, op=ALU.add)
        pfin = psum_pool.tile([P, CB], FP32, tag="ps", name="pfin")
        nc.tensor.matmul(pfin[:1, :1], lhsT=ones[:], rhs=rowtot[:])
        out_sb = epi_pool.tile([1, 1], FP32, tag="osb", name="out_sb")
        nc.vector.tensor_copy(out_sb[:], pfin[:1, :1])
        nc.sync.dma_start(out=out[:, :], in_=out_sb[:])

    with mock.patch("concourse.bacc.get_activation_tables",
                    _filtered_activation_tables):
        nc.compile()
    return nc


_CACHE_LOCK = threading.Lock()
_CACHED_NC = None


def _get_nc():
    global _CACHED_NC
    with _CACHE_LOCK:
        if _CACHED_NC is None:
            _CACHED_NC = _build_kernel()
        return _CACHED_NC


def _run(inputs, trace=False):
    z_i = np.asarray(inputs["z_i"], dtype=np.float32)
    z_j = np.asarray(inputs["z_j"], dtype=np.float32)
    reps = np.concatenate([z_i, z_j], axis=0)
    in_maps = [
        {"reps": np.ascontiguousarray(
            np.roll(reps, -ROWS_PER_CORE * i, axis=0))}
        for i in range(N_CORES)
    ]
    nc = _get_nc()
    res = run_bass_kernel_spmd(nc, in_maps, list(range(N_CORES)), trace=trace)
    partials = [float(res.results[i]["out"][0, 0]) for i in range(N_CORES)]
    loss = np.float32(np.sum(np.asarray(partials, dtype=np.float64)) / TWO_N)
    return loss, res


def kernel(**inputs):
    loss, _ = _run(inputs, trace=False)
    return np.asarray(loss, dtype=np.float32)


# revision 3
# speedup vs baseline: 2.3515x; 2.3515x over previous
"""NT-Xent / SimCLR contrastive loss on 8 Trainium2 NeuronCores.

Strategy (data-parallel over rows, fp8 DoubleRow matmuls):
  - Host: reps = concat(z_i, z_j) -> [8192, 512] fp32. Core i receives
    reps rolled by -1024*i rows so that *its* 1024 rows sit at rows 0..1023.
    SPMD program identical on every core; positive pairs at col = row+4096.
  - Device (per core), per 2048-row group, pipelined with phase B:
      phase A: load rows fp32 (Act hw queue); square+rowsum (DVE);
        inv = 16/||row|| via ln/exp (ACT, one table set); scale rows
        straight to fp8e4 (DVE/ACT split). Transpose WITHOUT any cast:
        adjacent fp8 byte-pairs are moved as uint16 elements through a
        DRAM bounce + xbar dma transpose (Sync hw queue), giving
        rT[kk][g][p2, q] = fp8 pair for d = 256*kk + 2*p2 + {0,1},
        permuted group column q (q = 16*j + t).
      phase B: DoubleRow fp8 matmuls contract logical k = (p2, i) pairs
        (labeling d = 256*kk + 2*p + i, identical on both operands), so
        the packed layout is consumed in place: psum[128,2048] = 256*sim.
        ACT exp(2*sim) with fused row-sum; DVE extracts self/positive
        diagonals via identity mask.
      epilogue: denom = rowsum - exp(2*sim_self); partial row loss is
        ln(denom) - 2*pos; partition-sum via ones-matmul; DMA scalar out.
  - Host: loss = sum(core partials) / 8192.
"""

import sys
import threading
from unittest import mock

sys.path.insert(0, "/opt/trn_rl_repo")

import numpy as np  # noqa: E402

import concourse.tile as tile  # noqa: E402
from concourse import bacc, mybir  # noqa: E402
from concourse.bass_utils import run_bass_kernel_spmd  # noqa: E402
from concourse.hw_specs import get_activation_tables  # noqa: E402
from concourse.masks import make_identity  # noqa: E402
from contextlib import ExitStack  # noqa: E402

P = 128
D = 512
TWO_N = 8192
N_CORES = 8
ROWS_PER_CORE = TWO_N // N_CORES  # 1024
T_INV = 2.0  # 1 / temperature (0.5)
S = 16.0  # fp8 pre-scale on normalized rows; psum = S*S*sim
SS = S * S

KK = 2  # DoubleRow calls (256 contraction each)
NB = 4  # big column blocks (= row groups in phase A)
CB = TWO_N // NB  # 2048 columns per block / rows per group
TPG = CB // P  # 16 [128, 512] row tiles per group
MB = ROWS_PER_CORE // P  # 8 m-blocks of 128 rows per core
NS = CB // 512  # 4 matmul sub-columns of 512 per block

FP32 = mybir.dt.float32
BF16 = mybir.dt.bfloat16
FP8 = mybir.dt.float8e4
U16 = mybir.dt.uint16
AF = mybir.ActivationFunctionType
ALU = mybir.AluOpType
AX = mybir.AxisListType
DR = mybir.MatmulPerfMode.DoubleRow


def _filtered_activation_tables(arch):
    """Steer every Exp/Ln/Copy activation to the one table set containing
    both Exp and Ln, so the table-load pass cannot thrash between sets."""
    tables = get_activation_tables(arch)
    target = None
    for name, funcs in tables.items():
        if AF.Exp in funcs and AF.Ln in funcs:
            target = name
            break
    if target is None:
        return tables
    steer = {AF.Exp, AF.Ln, AF.Copy, AF.Identity}
    return {
        name: (funcs if name == target else funcs - steer)
        for name, funcs in tables.items()
    }


def _build_kernel():
    nc = bacc.Bacc("TRN2", target_bir_lowering=False, debug=False,
                   num_devices=N_CORES)
    reps = nc.dram_tensor("reps", [TWO_N, D], FP32, kind="ExternalInput").ap()
    out = nc.dram_tensor("out", [1, 1], FP32, kind="ExternalOutput").ap()

    with tile.TileContext(nc) as tc, ExitStack() as ctx:
        rows_pool = ctx.enter_context(tc.tile_pool(name="rows", bufs=2))
        normed_pool = ctx.enter_context(tc.tile_pool(name="normed", bufs=2))
        sq_pool = ctx.enter_context(tc.tile_pool(name="sq", bufs=2))
        stats_pool = ctx.enter_context(tc.tile_pool(name="stats", bufs=1))
        repsT_pool = ctx.enter_context(tc.tile_pool(name="repsT", bufs=1))
        dram_pool = ctx.enter_context(
            tc.tile_pool(name="scratch", bufs=KK * 2, space="DRAM"))
        psum_pool = ctx.enter_context(
            tc.tile_pool(name="psum", bufs=2, space="PSUM"))
        exp_pool = ctx.enter_context(tc.tile_pool(name="exp", bufs=2))
        junk_pool = ctx.enter_context(tc.tile_pool(name="junk", bufs=2))
        epi_pool = ctx.enter_context(tc.tile_pool(name="epi", bufs=1))

        # --- constants -----------------------------------------------------
        ident = stats_pool.tile([P, P], FP32, tag="ident", name="ident")
        make_identity(nc, ident[:])
        ones = stats_pool.tile([P, 1], FP32, tag="ones", name="ones")
        nc.vector.memset(ones[:], 1.0)

        # accumulators for the main loop
        rs_all = stats_pool.tile([P, MB * NB], FP32, tag="rs", name="rs_all")
        e_self = stats_pool.tile([P, MB], FP32, tag="eself", name="e_self")
        pos = stats_pool.tile([P, MB], FP32, tag="pos", name="pos")

        # packed transposed tiles: rT[g][kk] [128, CB] uint16; element
        # [p2, q] packs fp8 values for d = 256*kk + 2*p2 + {0,1} of the
        # row at permuted group column q (q = 16*j + t <-> row t*128+j).
        rT = [[repsT_pool.tile([P, CB], U16, tag=f"rT_{g}_{kk}",
                               name=f"rT_{g}_{kk}")
               for kk in range(KK)]
              for g in range(NB)]
        # own 1024 columns un-permuted and unpacked into DoubleRow lhsT
        # layout: rT0[kk] [128, 2*1024] fp8; [p2, i*1024 + c] = value for
        # d = 256*kk + 2*p2 + i, own row c.
        rT0 = [repsT_pool.tile([P, KK * ROWS_PER_CORE], FP8, tag=f"rT0_{kk}",
                               name=f"rT0_{kk}")
               for kk in range(KK)]

        def phase_a(g):
            rows_g = rows_pool.tile([P, TPG * D], FP32, tag="rows",
                                    name=f"rows_{g}")
            src = reps[g * CB:(g + 1) * CB, :].rearrange(
                "(t p) d -> p t d", p=P)
            # input loads ride the Act hw queue; bounce rides Sync
            nc.scalar.dma_start(
                out=rows_g[:].rearrange("p (t d) -> p t d", d=D), in_=src)

            n2 = stats_pool.tile([P, TPG], FP32, tag="n2", bufs=2,
                                 name=f"n2_{g}")
            for t in range(TPG):
                sq = sq_pool.tile([P, D], BF16, tag="sq", name=f"sq_{g}_{t}")
                rt = rows_g[:, t * D:(t + 1) * D]
                nc.vector.scalar_tensor_tensor(
                    out=sq[:], in0=rt, scalar=1.0, in1=rt,
                    op0=ALU.mult, op1=ALU.mult, accum_out=n2[:, t:t + 1])
            # inv = S * n2**-0.5 = exp(-0.5 * ln(n2/S^2))
            lnn = stats_pool.tile([P, TPG], FP32, tag="lnn", bufs=2,
                                  name=f"lnn_{g}")
            nc.scalar.activation(lnn[:], n2[:], AF.Ln, scale=1.0 / SS)
            inv = stats_pool.tile([P, TPG], FP32, tag="inv", bufs=2,
                                  name=f"inv_{g}")
            nc.scalar.activation(inv[:], lnn[:], AF.Exp, scale=-0.5)

            normed_g = normed_pool.tile([P, TPG * D], FP8, tag="normed",
                                        name=f"normed_{g}")
            for t in range(TPG):
                src_t = rows_g[:, t * D:(t + 1) * D]
                dst_t = normed_g[:, t * D:(t + 1) * D]
                if t % 2 == 0:
                    nc.vector.tensor_scalar_mul(dst_t, src_t, inv[:, t:t + 1])
                else:
                    nc.scalar.activation(dst_t, src_t, AF.Copy,
                                         scale=inv[:, t:t + 1])
            # Bounce adjacent fp8 d-pairs as uint16 through DRAM, then xbar
            # transpose. scratch row q = p*16 + t holds row t*128 + p;
            # u16 column c2 of chunk kk = d-pair (256*kk + 2*c2, +1).
            nview = normed_g[:].bitcast(U16).rearrange(
                "p (t e) -> p t e", e=D // 2)
            for kk in range(KK):
                scr = dram_pool.tile([CB, P], U16, tag=f"scr{kk}",
                                     name=f"scr_{kk}_{g}")
                nc.sync.dma_start(
                    out=scr[:].rearrange("(p t) c -> p t c", p=P),
                    in_=nview[:, :, kk * P:(kk + 1) * P])
                nc.sync.dma_start_transpose(rT[g][kk][:], scr[:])
                if g == 0:
                    # unpack + un-permute own columns for contiguous lhsT:
                    # in [p2, (j m i)] (fp8 flat = 32j + 2m + i, m<8)
                    # -> out [p2, (i m j)]
                    src8 = rT[g][kk][:].bitcast(FP8).rearrange(
                        "p (j m i) -> p i m j", i=2, m=TPG)[:, :, :MB, :]
                    dst8 = rT0[kk][:].rearrange(
                        "p (i m j) -> p i m j", i=2, m=MB)
                    nc.vector.tensor_copy(dst8, src8)

        # permuted-column selector: columns for m-block rows m*128..m*128+127
        # sit at positions 16*j + m (j = psum row).
        def colsel(ap_2d, m):
            return ap_2d.rearrange("p (j s) -> p s j", s=TPG)[:, m, :]

        def phase_b(nb):
            for m in range(MB):
                ps = psum_pool.tile([P, CB], FP32, tag="ps",
                                    name=f"ps_{nb}_{m}")
                for kk in range(KK):
                    lhsT = rT0[kk][:].rearrange(
                        "p (i c) -> p i c", i=KK)[:, :, m * P:(m + 1) * P]
                    rview = rT[nb][kk][:].bitcast(FP8).rearrange(
                        "p (q i) -> p i q", i=2)
                    for ns in range(NS):
                        nc.tensor.matmul(
                            ps[:, ns * 512:(ns + 1) * 512],
                            lhsT=lhsT,
                            rhs=rview[:, :, ns * 512:(ns + 1) * 512],
                            start=(kk == 0), stop=(kk == KK - 1),
                            perf_mode=DR)
                et = exp_pool.tile([P, CB], BF16, tag="et", name=f"et_{nb}_{m}")
                nc.scalar.activation(
                    et[:], ps[:], AF.Exp, scale=T_INV / SS,
                    accum_out=rs_all[:, m * NB + nb:m * NB + nb + 1])
                if nb == 0:
                    # self-similarity column: global col = row = m*128 + j,
                    # at permuted position 16*j + m.
                    junk = junk_pool.tile([P, P], FP32, tag="junk",
                                          name=f"junk_s_{m}")
                    nc.vector.scalar_tensor_tensor(
                        out=junk[:], in0=colsel(et[:], m),
                        scalar=1.0, in1=ident[:],
                        op0=ALU.mult, op1=ALU.mult,
                        accum_out=e_self[:, m:m + 1])
                if nb == 2:
                    # positive column: global col = 4096 + row, in-group
                    # offset = row -> same permuted position 16*j + m.
                    junk = junk_pool.tile([P, P], FP32, tag="junk",
                                          name=f"junk_p_{m}")
                    nc.vector.scalar_tensor_tensor(
                        out=junk[:], in0=colsel(ps[:], m),
                        scalar=1.0, in1=ident[:],
                        op0=ALU.mult, op1=ALU.mult,
                        accum_out=pos[:, m:m + 1])

        # interleave: phase B for group g right after its phase A, so the
        # tile scheduler can overlap group g+1's loads with group g's mms.
        for g in range(NB):
            phase_a(g)
            phase_b(g)

        # --- epilogue ------------------------------------------------------
        sums = epi_pool.tile([P, MB], FP32, tag="sums", name="sums")
        nc.vector.tensor_reduce(
            sums[:], rs_all[:].rearrange("p (m b) -> p m b", b=NB),
            axis=AX.X, op=ALU.add)
        denom = epi_pool.tile([P, MB], FP32, tag="denom", name="denom")
        nc.vector.tensor_sub(denom[:], sums[:], e_self[:])
        ld = epi_pool.tile([P, MB], FP32, tag="ld", name="ld")
        nc.scalar.activation(ld[:], denom[:], AF.Ln)
        # partial = ld - (2/SS)*pos   (pos holds 256*sim)
        part = epi_pool.tile([P, MB], FP32, tag="part", name="part")
        nc.vector.scalar_tensor_tensor(
            out=part[:], in0=pos[:], scalar=-T_INV / SS, in1=ld[:],
            op0=ALU.mult, op1=ALU.add)
        rowtot = epi_pool.tile([P, 1], FP32, tag="rowtot", name="rowtot")
        nc.vector.tensor_reduce(rowtot[:], part[:], axis=AX.X, op=ALU.add)
        pfin = psum_pool.tile([P, CB], FP32, tag="ps", name="pfin")
        nc.tensor.matmul(pfin[:1, :1], lhsT=ones[:], rhs=rowtot[:])
        out_sb = epi_pool.tile([1, 1], FP32, tag="osb", name="out_sb")
        nc.vector.tensor_copy(out_sb[:], pfin[:1, :1])
        nc.sync.dma_start(out=out[:, :], in_=out_sb[:])

    with mock.patch("concourse.bacc.get_activation_tables",
                    _filtered_activation_tables):
        nc.compile()
    return nc


_CACHE_LOCK = threading.Lock()
_CACHED_NC = None


def _get_nc():
    global _CACHED_NC
    with _CACHE_LOCK:
        if _CACHED_NC is None:
            _CACHED_NC = _build_kernel()
        return _CACHED_NC


def _run(inputs, trace=False):
    z_i = np.asarray(inputs["z_i"], dtype=np.float32)
    z_j = np.asarray(inputs["z_j"], dtype=np.float32)
    reps = np.concatenate([z_i, z_j], axis=0)
    in_maps = [
        {"reps": np.ascontiguousarray(
            np.roll(reps, -ROWS_PER_CORE * i, axis=0))}
        for i in range(N_CORES)
    ]
    nc = _get_nc()
    res = run_bass_kernel_spmd(nc, in_maps, list(range(N_CORES)), trace=trace)
    partials = [float(res.results[i]["out"][0, 0]) for i in range(N_CORES)]
    loss = np.float32(np.sum(np.asarray(partials, dtype=np.float64)) / TWO_N)
    return loss, res


def kernel(**inputs):
    loss, _ = _run(inputs, trace=False)
    return np.asarray(loss, dtype=np.float32)


# revision 9
# speedup vs baseline: 2.4854x; 1.0569x over previous
"""NT-Xent / SimCLR contrastive loss on 8 Trainium2 NeuronCores.

Symmetric-halved data-parallel scheme with fp8 DoubleRow matmuls:
  - Host: reps = concat(z_i, z_j) -> [8192, 512] fp32. Core c receives
    rows (1024c .. 1024c+5120) mod 8192 ("rolled"), so its own 1024 rows
    sit at 0..1023 and it computes its [1024, 5120] slice of the
    similarity matrix: distance blocks d=0 (diag, incl. self), 1, 2, 3
    and d=4 (holds the positives at col = row + 4096).
  - sim is symmetric: pairs at distance 5..7 from core c are distance
    1..3 from another core. Each core column-sums its exp blocks d=1..3
    (ones-vector matmuls; the contributions its exp values owe to the
    block-owning cores' denominators); the [3*1024] vectors are
    AllGathered and each core picks its 3 incoming chunks with a
    host-supplied 0/1 mask (pure sharding metadata) and adds them to its
    row sums. Only the distance-4 block is computed twice (8% overhead
    instead of full mirroring's 60%).
  - Row r of a group lives at (partition, tile) = (r // 8, r % 8), so
    input loads are 128 contiguous 16 KiB descriptors and the transposed
    column order is the identity (no permutations anywhere).
  - Per 1024-row group: square+rowsum (DVE), inv = 16/||row|| (ACT
    ln/exp, one table set), scale rows straight to fp8e4 (DVE).
    Transpose with no cast: adjacent fp8 byte-pairs travel as uint16
    through a DRAM bounce + xbar dma transpose (Sync hw queue).
    DoubleRow matmuls contract logical k = (p2, i) (d = 256*kk + 2*p2
    + i, same labeling both operands) so the packed layout is consumed
    in place. ACT exp(2*sim) with fused row-sum; DVE extracts
    self/positive diagonals with an identity mask.
  - denom = rowsums + gathered colsums - exp(2*sim_self); row loss =
    ln(denom) - 2*pos; partition-sum via ones-matmul; host sums the 8
    partials / 8192.
"""

import sys
import threading
from unittest import mock

sys.path.insert(0, "/opt/trn_rl_repo")

import numpy as np  # noqa: E402

import concourse.tile as tile  # noqa: E402
from concourse import bacc, mybir  # noqa: E402
from concourse.bass_utils import run_bass_kernel_spmd  # noqa: E402
from concourse.hw_specs import get_activation_tables  # noqa: E402
from concourse.masks import make_identity  # noqa: E402
from contextlib import ExitStack  # noqa: E402

P = 128
D = 512
TWO_N = 8192
N_CORES = 8
ROWS_PER_CORE = TWO_N // N_CORES  # 1024
T_INV = 2.0  # 1 / temperature (0.5)
S = 16.0  # fp8 pre-scale on normalized rows; psum = S*S*sim
SS = S * S

KK = 2  # DoubleRow calls (256 contraction each)
NBLK = 5  # distance blocks computed per core
GB = ROWS_PER_CORE  # 1024 rows per group / cols per block
NROWS = NBLK * GB  # 5120 rows loaded per core
TPG = GB // P  # 8 [128, 512] row tiles per group
MB = ROWS_PER_CORE // P  # 8 m-blocks of 128 rows per core
NS = GB // 512  # 2 matmul sub-columns of 512 per block
NCS = 3  # blocks whose colsums are exchanged (d=1..3)

FP32 = mybir.dt.float32
BF16 = mybir.dt.bfloat16
FP8 = mybir.dt.float8e4
U16 = mybir.dt.uint16
AF = mybir.ActivationFunctionType
ALU = mybir.AluOpType
AX = mybir.AxisListType
DR = mybir.MatmulPerfMode.DoubleRow


def _filtered_activation_tables(arch):
    """Steer every Exp/Ln/Copy activation to the one table set containing
    both Exp and Ln, so the table-load pass cannot thrash between sets."""
    tables = get_activation_tables(arch)
    target = None
    for name, funcs in tables.items():
        if AF.Exp in funcs and AF.Ln in funcs:
            target = name
            break
    if target is None:
        return tables
    steer = {AF.Exp, AF.Ln, AF.Copy, AF.Identity}
    return {
        name: (funcs if name == target else funcs - steer)
        for name, funcs in tables.items()
    }


def _build_kernel():
    nc = bacc.Bacc("TRN2", target_bir_lowering=False, debug=False,
                   num_devices=N_CORES)
    reps = nc.dram_tensor("reps", [NROWS, D], FP32, kind="ExternalInput").ap()
    cmask = nc.dram_tensor("cmask", [P, N_CORES * NCS * MB], FP32,
                           kind="ExternalInput").ap()
    out = nc.dram_tensor("out", [1, 1], FP32, kind="ExternalOutput").ap()

    with tile.TileContext(nc) as tc, ExitStack() as ctx:
        rows_pool = ctx.enter_context(tc.tile_pool(name="rows", bufs=NBLK))
        normed_pool = ctx.enter_context(tc.tile_pool(name="normed", bufs=2))
        sq_pool = ctx.enter_context(tc.tile_pool(name="sq", bufs=2))
        stats_pool = ctx.enter_context(tc.tile_pool(name="stats", bufs=1))
        repsT_pool = ctx.enter_context(tc.tile_pool(name="repsT", bufs=1))
        dram_pool = ctx.enter_context(
            tc.tile_pool(name="scratch", bufs=KK * 2, space="DRAM"))
        cc_pool = ctx.enter_context(
            tc.tile_pool(name="ccdram", bufs=1, space="DRAM"))
        psum_pool = ctx.enter_context(
            tc.tile_pool(name="psum", bufs=2, space="PSUM"))
        cs_pool = ctx.enter_context(
            tc.tile_pool(name="cspsum", bufs=2 * NS, space="PSUM"))
        exp_pool = ctx.enter_context(tc.tile_pool(name="exp", bufs=4))
        junk_pool = ctx.enter_context(tc.tile_pool(name="junk", bufs=2))
        epi_pool = ctx.enter_context(tc.tile_pool(name="epi", bufs=1))

        # --- input loads, all dispatched up front (Act hw queue) -----------
        # row r of group g lives at (partition r//8, tile r%8): the load is
        # 128 contiguous 16 KiB runs.
        rows_tiles = []
        for g in range(NBLK):
            rows_g = rows_pool.tile([P, TPG * D], FP32, tag="rows",
                                    name=f"rows_{g}")
            src = reps[g * GB:(g + 1) * GB, :].rearrange(
                "(p t) d -> p t d", p=P)
            nc.scalar.dma_start(
                out=rows_g[:].rearrange("p (t d) -> p t d", d=D), in_=src)
            rows_tiles.append(rows_g)
        cmask_sb = stats_pool.tile([P, N_CORES * NCS * MB], FP32,
                                   tag="cmask", name="cmask_sb")
        nc.sync.dma_start(out=cmask_sb[:], in_=cmask[:, :])

        # --- constants -----------------------------------------------------
        ident = stats_pool.tile([P, P], FP32, tag="ident", name="ident")
        make_identity(nc, ident[:])
        ones = stats_pool.tile([P, 1], FP32, tag="ones", name="ones")
        nc.vector.memset(ones[:], 1.0)
        ones_bf = stats_pool.tile([P, 1], BF16, tag="onesb", name="ones_bf")
        nc.vector.memset(ones_bf[:], 1.0)

        # accumulators, all in psum row layout: [j, m] = row m*128 + j
        rs_all = stats_pool.tile([P, MB * NBLK], FP32, tag="rs",
                                 name="rs_all")
        e_self = stats_pool.tile([P, MB], FP32, tag="eself", name="e_self")
        pos = stats_pool.tile([P, MB], FP32, tag="pos", name="pos")
        # colsum staging, already permuted for the receivers' readback:
        # position (d-1)*1024 + 8*j + m holds the contribution to the
        # target core's row m*128 + j.
        csums = stats_pool.tile([1, NCS * GB], FP32, tag="csums",
                                name="csums")

        # packed transposed tiles: rT[g][kk] [128, 1024] uint16; element
        # [p2, q] packs fp8 values for d = 256*kk + 2*p2 + {0,1} of
        # group row q.
        rT = [[repsT_pool.tile([P, GB], U16, tag=f"rT_{g}_{kk}",
                               name=f"rT_{g}_{kk}")
               for kk in range(KK)]
              for g in range(NBLK)]
        # own columns unpacked into DoubleRow lhsT layout: rT0[kk]
        # [128, 2*1024] fp8; [p2, i*1024 + c] = d = 256kk + 2p2 + i, row c.
        rT0 = [repsT_pool.tile([P, KK * ROWS_PER_CORE], FP8, tag=f"rT0_{kk}",
                               name=f"rT0_{kk}")
               for kk in range(KK)]

        # DRAM tiles for the colsum exchange (flat on one partition)
        snd = cc_pool.tile([1, NCS * GB], FP32, name="snd")
        gat = cc_pool.tile([N_CORES, NCS * GB], FP32, name="gat")

        def phase_a(g):
            rows_g = rows_tiles[g]
            n2 = stats_pool.tile([P, TPG], FP32, tag="n2", bufs=2,
                                 name=f"n2_{g}")
            for t in range(TPG):
                sq = sq_pool.tile([P, D], BF16, tag="sq", name=f"sq_{g}_{t}")
                rt = rows_g[:, t * D:(t + 1) * D]
                nc.vector.scalar_tensor_tensor(
                    out=sq[:], in0=rt, scalar=1.0, in1=rt,
                    op0=ALU.mult, op1=ALU.mult, accum_out=n2[:, t:t + 1])
            # inv = S * n2**-0.5 = exp(-0.5 * ln(n2/S^2))
            lnn = stats_pool.tile([P, TPG], FP32, tag="lnn", bufs=2,
                                  name=f"lnn_{g}")
            nc.scalar.activation(lnn[:], n2[:], AF.Ln, scale=1.0 / SS)
            inv = stats_pool.tile([P, TPG], FP32, tag="inv", bufs=2,
                                  name=f"inv_{g}")
            nc.scalar.activation(inv[:], lnn[:], AF.Exp, scale=-0.5)

            normed_g = normed_pool.tile([P, TPG * D], FP8, tag="normed",
                                        name=f"normed_{g}")
            for t in range(TPG):
                nc.vector.tensor_scalar_mul(
                    normed_g[:, t * D:(t + 1) * D],
                    rows_g[:, t * D:(t + 1) * D], inv[:, t:t + 1])
            # Bounce adjacent fp8 d-pairs as uint16 through DRAM, then xbar
            # transpose. scratch row q = 8p + t = group row q.
            nview = normed_g[:].bitcast(U16).rearrange(
                "p (t e) -> p t e", e=D // 2)
            for kk in range(KK):
                scr = dram_pool.tile([GB, P], U16, tag=f"scr{kk}",
                                     name=f"scr_{kk}_{g}")
                nc.sync.dma_start(
                    out=scr[:].rearrange("(p t) c -> p t c", p=P),
                    in_=nview[:, :, kk * P:(kk + 1) * P])
                nc.sync.dma_start_transpose(rT[g][kk][:], scr[:])
                if g == 0:
                    # unpack own columns for contiguous lhsT
                    src8 = rT[g][kk][:].bitcast(FP8).rearrange(
                        "p (c i) -> p i c", i=2)
                    dst8 = rT0[kk][:].rearrange(
                        "p (i c) -> p i c", i=2)
                    nc.vector.tensor_copy(dst8, src8)

        def cs_mm(cs, m, et):
            for ns in range(NS):
                nc.tensor.matmul(
                    cs[ns][:, :],
                    lhsT=ones_bf[:],
                    rhs=et[:, ns * 512:(ns + 1) * 512],
                    start=(m == 0), stop=(m == MB - 1))

        def phase_b(nb):
            do_cs = 1 <= nb <= NCS
            cs = None
            ets = {}
            if do_cs:
                cs = [cs_pool.tile([1, 512], FP32, tag="cs",
                                   name=f"cs_{nb}_{ns}")
                      for ns in range(NS)]
            for m in range(MB):
                ps = psum_pool.tile([P, GB], FP32, tag="ps",
                                    name=f"ps_{nb}_{m}")
                for kk in range(KK):
                    lhsT = rT0[kk][:].rearrange(
                        "p (i c) -> p i c", i=KK)[:, :, m * P:(m + 1) * P]
                    rview = rT[nb][kk][:].bitcast(FP8).rearrange(
                        "p (q i) -> p i q", i=2)
                    for ns in range(NS):
                        nc.tensor.matmul(
                            ps[:, ns * 512:(ns + 1) * 512],
                            lhsT=lhsT,
                            rhs=rview[:, :, ns * 512:(ns + 1) * 512],
                            start=(kk == 0), stop=(kk == KK - 1),
                            perf_mode=DR)
                et = exp_pool.tile([P, GB], BF16, tag="et",
                                   name=f"et_{nb}_{m}")
                nc.scalar.activation(
                    et[:], ps[:], AF.Exp, scale=T_INV / SS,
                    accum_out=rs_all[:, m * NBLK + nb:m * NBLK + nb + 1])
                if do_cs:
                    # colsum matmuls lag one m step so the PE never stalls
                    # on the ACT exp of the current tile
                    ets[m] = et
                    if m > 0:
                        cs_mm(cs, m - 1, ets.pop(m - 1)[:])
                if nb == 0:
                    # self-similarity: col = row = m*128 + j
                    junk = junk_pool.tile([P, P], FP32, tag="junk",
                                          name=f"junk_s_{m}")
                    nc.vector.scalar_tensor_tensor(
                        out=junk[:], in0=et[:, m * P:(m + 1) * P],
                        scalar=1.0, in1=ident[:],
                        op0=ALU.mult, op1=ALU.mult,
                        accum_out=e_self[:, m:m + 1])
                if nb == NBLK - 1:
                    # positive: col = 4096 + row -> block 4 offset = row
                    junk = junk_pool.tile([P, P], FP32, tag="junk",
                                          name=f"junk_p_{m}")
                    nc.vector.scalar_tensor_tensor(
                        out=junk[:], in0=ps[:, m * P:(m + 1) * P],
                        scalar=1.0, in1=ident[:],
                        op0=ALU.mult, op1=ALU.mult,
                        accum_out=pos[:, m:m + 1])
            if do_cs:
                cs_mm(cs, MB - 1, ets.pop(MB - 1)[:])
                # stage into csums permuted: cs[ns][0, q] (q = m*128 + j)
                # -> csums[0, (nb-1)*1024 + 8j + m]
                base = (nb - 1) * GB
                mpc = 512 // P  # m-blocks covered per cs chunk
                for ns in range(NS):
                    dst = csums[:, base:base + GB].rearrange(
                        "o (j m) -> o m j", m=MB)[:, mpc * ns:mpc * (ns + 1), :]
                    src = cs[ns][:, :].rearrange("o (m j) -> o m j", j=P)
                    nc.vector.tensor_copy(dst, src)

        # order: group 0 first (lhsT), then the colsum blocks 1..3, kick
        # off the AllGather, then block 4 overlaps the collective.
        for g in range(NBLK):
            phase_a(g)
            phase_b(g)
            if g == NCS:
                nc.sync.dma_start(out=snd[:], in_=csums[:])
                nc.gpsimd.collective_compute(
                    "AllGather", ALU.bypass,
                    replica_groups=[list(range(N_CORES))],
                    ins=[snd[:].opt()], outs=[gat[:].opt()])

        # pull the gathered [8, 3072] back; slot s = src*3 + (d-1); the
        # sender already permuted so position 8j + m = our row m*128 + j.
        gat_sb = epi_pool.tile([P, N_CORES * NCS * MB], FP32, tag="gat",
                               name="gat_sb")
        nc.sync.dma_start(
            out=gat_sb[:].rearrange("j (r c m) -> j r c m", r=N_CORES,
                                    c=NCS),
            in_=gat[:].rearrange("r (c j m) -> j r c m", c=NCS, j=P))

        # extra[j, m] = sum_s cmask[s] * gat_sb[j, s, m]
        masked = epi_pool.tile([P, N_CORES * NCS * MB], FP32, tag="msk",
                               name="masked")
        nc.vector.tensor_mul(masked[:], gat_sb[:], cmask_sb[:])
        extra = epi_pool.tile([P, MB], FP32, tag="extra", name="extra")
        nc.vector.tensor_reduce(
            extra[:], masked[:].rearrange("j (s m) -> j m s", m=MB),
            axis=AX.X, op=ALU.add)

        # --- epilogue ------------------------------------------------------
        sums = epi_pool.tile([P, MB], FP32, tag="sums", name="sums")
        nc.vector.tensor_reduce(
            sums[:], rs_all[:].rearrange("p (m b) -> p m b", b=NBLK),
            axis=AX.X, op=ALU.add)
        sums2 = epi_pool.tile([P, MB], FP32, tag="sums2", name="sums2")
        nc.vector.tensor_add(sums2[:], sums[:], extra[:])
        denom = epi_pool.tile([P, MB], FP32, tag="denom", name="denom")
        nc.vector.tensor_sub(denom[:], sums2[:], e_self[:])
        ld = epi_pool.tile([P, MB], FP32, tag="ld", name="ld")
        nc.scalar.activation(ld[:], denom[:], AF.Ln)
        # partial = ld - (2/SS)*pos   (pos holds 256*sim)
        part = epi_pool.tile([P, MB], FP32, tag="part", name="part")
        nc.vector.scalar_tensor_tensor(
            out=part[:], in0=pos[:], scalar=-T_INV / SS, in1=ld[:],
            op0=ALU.mult, op1=ALU.add)
        rowtot = epi_pool.tile([P, 1], FP32, tag="rowtot", name="rowtot")
        nc.vector.tensor_reduce(rowtot[:], part[:], axis=AX.X, op=ALU.add)
        pfin = psum_pool.tile([P, GB], FP32, tag="ps", name="pfin")
        nc.tensor.matmul(pfin[:1, :1], lhsT=ones[:], rhs=rowtot[:])
        out_sb = epi_pool.tile([1, 1], FP32, tag="osb", name="out_sb")
        nc.vector.tensor_copy(out_sb[:], pfin[:1, :1])
        nc.sync.dma_start(out=out[:, :], in_=out_sb[:])

    with mock.patch("concourse.bacc.get_activation_tables",
                    _filtered_activation_tables):
        nc.compile()
    return nc


_CACHE_LOCK = threading.Lock()
_CACHED_NC = None


def _get_nc():
    global _CACHED_NC
    with _CACHE_LOCK:
        if _CACHED_NC is None:
            _CACHED_NC = _build_kernel()
        return _CACHED_NC


def _make_cmask(c):
    """0/1 pick of the 3 gathered colsum chunks core c needs: slot
    s = src*3 + (d-1) is wanted iff src == (c - d) mod 8."""
    m = np.zeros(N_CORES * NCS, np.float32)
    for d in range(1, NCS + 1):
        src = (c - d) % N_CORES
        m[src * NCS + (d - 1)] = 1.0
    m = np.repeat(m, MB)  # [24*8], s-major then m
    return np.broadcast_to(m, (P, m.size)).copy()


def _run(inputs, trace=False):
    z_i = np.asarray(inputs["z_i"], dtype=np.float32)
    z_j = np.asarray(inputs["z_j"], dtype=np.float32)
    reps = np.concatenate([z_i, z_j], axis=0)
    in_maps = []
    for c in range(N_CORES):
        rolled = np.roll(reps, -ROWS_PER_CORE * c, axis=0)[:NROWS]
        in_maps.append({
            "reps": np.ascontiguousarray(rolled),
            "cmask": _make_cmask(c),
        })
    nc = _get_nc()
    res = run_bass_kernel_spmd(nc, in_maps, list(range(N_CORES)), trace=trace)
    partials = [float(res.results[i]["out"][0, 0]) for i in range(N_CORES)]
    loss = np.float32(np.sum(np.asarray(partials, dtype=np.float64)) / TWO_N)
    return loss, res


def kernel(**inputs):
    loss, _ = _run(inputs, trace=False)
    return np.asarray(loss, dtype=np.float32)


# revision 11
# speedup vs baseline: 2.5806x; 1.0383x over previous
"""NT-Xent / SimCLR contrastive loss on 8 Trainium2 NeuronCores.

Symmetric-halved data-parallel scheme with fp8 DoubleRow matmuls:
  - Host: reps = concat(z_i, z_j) -> [8192, 512] fp32. Core c receives
    rows (1024c .. 1024c+5120) mod 8192 ("rolled"), so its own 1024 rows
    sit at 0..1023 and it computes its [1024, 5120] slice of the
    similarity matrix: distance blocks d=0 (diag, incl. self), 1, 2, 3
    and d=4 (holds the positives at col = row + 4096).
  - sim is symmetric: pairs at distance 5..7 from core c are distance
    1..3 from another core. Each core column-sums its exp blocks d=1..3
    (ones-vector matmuls); the [3*1024] vectors are AllGathered and each
    core picks its 3 incoming chunks with a host-supplied 0/1 mask (pure
    sharding metadata) and adds them to its row sums. Only the
    distance-4 block is computed twice (8% overhead instead of full
    mirroring's 60%). Blocks are processed 1,2,3 -> AllGather -> 4,0 so
    the collective latency hides behind two blocks of compute.
  - Row r of a group lives at (partition, tile) = (r // 8, r % 8):
    loads are 128 contiguous 16 KiB descriptors and transposed column
    order is the identity. Phase A is pipelined at 512-row half-group
    granularity to shorten the startup fill.
  - Per half-group: square+rowsum (DVE), inv = 16/||row|| (ACT ln/exp,
    one table set), scale rows straight to fp8e4 (DVE). Transpose with
    no cast: adjacent fp8 byte-pairs travel as uint16 through a DRAM
    bounce + xbar dma transpose (Sync hw queue). DoubleRow matmuls
    contract logical k = (p2, i) (d = 256*kk + 2*p2 + i on both
    operands) so the packed layout is consumed in place. ACT exp(2*sim)
    with fused row-sum; DVE extracts self/positive diagonals.
  - denom = rowsums + gathered colsums - exp(2*sim_self); row loss =
    ln(denom) - 2*pos; partition-sum via ones-matmul; host sums the 8
    partials / 8192.
"""

import sys
import threading
from unittest import mock

sys.path.insert(0, "/opt/trn_rl_repo")

import numpy as np  # noqa: E402

import concourse.tile as tile  # noqa: E402
from concourse import bacc, mybir  # noqa: E402
from concourse.bass_utils import run_bass_kernel_spmd  # noqa: E402
from concourse.hw_specs import get_activation_tables  # noqa: E402
from concourse.masks import make_identity  # noqa: E402
from contextlib import ExitStack  # noqa: E402

P = 128
D = 512
TWO_N = 8192
N_CORES = 8
ROWS_PER_CORE = TWO_N // N_CORES  # 1024
T_INV = 2.0  # 1 / temperature (0.5)
S = 16.0  # fp8 pre-scale on normalized rows; psum = S*S*sim
SS = S * S

KK = 2  # DoubleRow calls (256 contraction each)
NBLK = 5  # distance blocks computed per core
GB = ROWS_PER_CORE  # 1024 rows per group / cols per block
NROWS = NBLK * GB  # 5120 rows loaded per core
HG = 512  # half-group rows (phase A pipeline granularity)
THG = HG // P  # 4 row tiles per half-group
MB = ROWS_PER_CORE // P  # 8 m-blocks of 128 rows per core
NS = GB // 512  # 2 matmul sub-columns of 512 per block
NCS = 3  # blocks whose colsums are exchanged (d=1..3)

FP32 = mybir.dt.float32
BF16 = mybir.dt.bfloat16
FP8 = mybir.dt.float8e4
U16 = mybir.dt.uint16
AF = mybir.ActivationFunctionType
ALU = mybir.AluOpType
AX = mybir.AxisListType
DR = mybir.MatmulPerfMode.DoubleRow

BLOCK_ORDER = [1, 2, 3, 4, 0]  # collective kicks after 3; 4 and 0 hide it


def _filtered_activation_tables(arch):
    """Steer every Exp/Ln/Copy activation to the one table set containing
    both Exp and Ln, so the table-load pass cannot thrash between sets."""
    tables = get_activation_tables(arch)
    target = None
    for name, funcs in tables.items():
        if AF.Exp in funcs and AF.Ln in funcs:
            target = name
            break
    if target is None:
        return tables
    steer = {AF.Exp, AF.Ln, AF.Copy, AF.Identity}
    return {
        name: (funcs if name == target else funcs - steer)
        for name, funcs in tables.items()
    }


def _build_kernel():
    nc = bacc.Bacc("TRN2", target_bir_lowering=False, debug=False,
                   num_devices=N_CORES)
    reps = nc.dram_tensor("reps", [NROWS, D], FP32, kind="ExternalInput").ap()
    cmask = nc.dram_tensor("cmask", [P, N_CORES * NCS * MB], FP32,
                           kind="ExternalInput").ap()
    out = nc.dram_tensor("out", [1, 1], FP32, kind="ExternalOutput").ap()

    n_halves = NROWS // HG  # 10

    with tile.TileContext(nc) as tc, ExitStack() as ctx:
        rows_pool = ctx.enter_context(
            tc.tile_pool(name="rows", bufs=n_halves))
        normed_pool = ctx.enter_context(tc.tile_pool(name="normed", bufs=3))
        sq_pool = ctx.enter_context(tc.tile_pool(name="sq", bufs=2))
        stats_pool = ctx.enter_context(tc.tile_pool(name="stats", bufs=1))
        repsT_pool = ctx.enter_context(tc.tile_pool(name="repsT", bufs=1))
        dram_pool = ctx.enter_context(
            tc.tile_pool(name="scratch", bufs=KK * 3, space="DRAM"))
        cc_pool = ctx.enter_context(
            tc.tile_pool(name="ccdram", bufs=1, space="DRAM"))
        psum_pool = ctx.enter_context(
            tc.tile_pool(name="psum", bufs=3, space="PSUM"))
        cs_pool = ctx.enter_context(
            tc.tile_pool(name="cspsum", bufs=NS, space="PSUM"))
        exp_pool = ctx.enter_context(tc.tile_pool(name="exp", bufs=4))
        junk_pool = ctx.enter_context(tc.tile_pool(name="junk", bufs=2))
        epi_pool = ctx.enter_context(tc.tile_pool(name="epi", bufs=1))

        # --- input loads, all dispatched up front (Act hw queue) -----------
        # rows of half h: reps[512h + (4p + t)] at (partition p, tile t):
        # 128 contiguous 8 KiB descriptors per load.
        rows_tiles = []
        for h in range(n_halves):
            rows_h = rows_pool.tile([P, THG * D], FP32, tag="rows",
                                    name=f"rows_{h}")
            src = reps[h * HG:(h + 1) * HG, :].rearrange(
                "(p t) d -> p t d", p=P)
            nc.scalar.dma_start(
                out=rows_h[:].rearrange("p (t d) -> p t d", d=D), in_=src)
            rows_tiles.append(rows_h)
        cmask_sb = stats_pool.tile([P, N_CORES * NCS * MB], FP32,
                                   tag="cmask", name="cmask_sb")
        nc.sync.dma_start(out=cmask_sb[:], in_=cmask[:, :])

        # --- constants -----------------------------------------------------
        ident = stats_pool.tile([P, P], FP32, tag="ident", name="ident")
        make_identity(nc, ident[:])
        ones = stats_pool.tile([P, 1], FP32, tag="ones", name="ones")
        nc.vector.memset(ones[:], 1.0)
        ones_bf = stats_pool.tile([P, 1], BF16, tag="onesb", name="ones_bf")
        nc.vector.memset(ones_bf[:], 1.0)

        # accumulators, all in psum row layout: [j, m] = row m*128 + j
        rs_all = stats_pool.tile([P, MB * NBLK], FP32, tag="rs",
                                 name="rs_all")
        e_self = stats_pool.tile([P, MB], FP32, tag="eself", name="e_self")
        pos = stats_pool.tile([P, MB], FP32, tag="pos", name="pos")
        # colsum staging, permuted for the receivers' readback: position
        # (d-1)*1024 + 8j + m holds the contribution to target row m*128+j
        csums = stats_pool.tile([1, NCS * GB], FP32, tag="csums",
                                name="csums")

        # packed transposed tiles: rT[g][kk] [128, 1024] uint16; element
        # [p2, q] packs fp8 values for d = 256*kk + 2*p2 + {0,1} of
        # group row q.
        rT = [[repsT_pool.tile([P, GB], U16, tag=f"rT_{g}_{kk}",
                               name=f"rT_{g}_{kk}")
               for kk in range(KK)]
              for g in range(NBLK)]
        # own columns unpacked into DoubleRow lhsT layout: rT0[kk]
        # [128, 2*1024] fp8; [p2, i*1024 + c] = d = 256kk + 2p2 + i, row c.
        rT0 = [repsT_pool.tile([P, KK * ROWS_PER_CORE], FP8, tag=f"rT0_{kk}",
                               name=f"rT0_{kk}")
               for kk in range(KK)]

        # DRAM tiles for the colsum exchange (flat on one partition)
        snd = cc_pool.tile([1, NCS * GB], FP32, name="snd")
        gat = cc_pool.tile([N_CORES, NCS * GB], FP32, name="gat")

        def phase_a_half(h):
            g, hh = h // 2, h % 2
            rows_h = rows_tiles[h]
            n2 = stats_pool.tile([P, THG], FP32, tag="n2", bufs=3,
                                 name=f"n2_{h}")
            for t in range(THG):
                sq = sq_pool.tile([P, D], BF16, tag="sq", name=f"sq_{h}_{t}")
                rt = rows_h[:, t * D:(t + 1) * D]
                nc.vector.scalar_tensor_tensor(
                    out=sq[:], in0=rt, scalar=1.0, in1=rt,
                    op0=ALU.mult, op1=ALU.mult, accum_out=n2[:, t:t + 1])
            # inv = S * n2**-0.5 = exp(-0.5 * ln(n2/S^2))
            lnn = stats_pool.tile([P, THG], FP32, tag="lnn", bufs=3,
                                  name=f"lnn_{h}")
            nc.scalar.activation(lnn[:], n2[:], AF.Ln, scale=1.0 / SS)
            inv = stats_pool.tile([P, THG], FP32, tag="inv", bufs=3,
                                  name=f"inv_{h}")
            nc.scalar.activation(inv[:], lnn[:], AF.Exp, scale=-0.5)

            normed_h = normed_pool.tile([P, THG * D], FP8, tag="normed",
                                        name=f"normed_{h}")
            for t in range(THG):
                nc.vector.tensor_scalar_mul(
                    normed_h[:, t * D:(t + 1) * D],
                    rows_h[:, t * D:(t + 1) * D], inv[:, t:t + 1])
            # Bounce fp8 d-pairs as uint16 through DRAM, then xbar
            # transpose into columns [512*hh, 512*hh+512) of rT[g][kk].
            nview = normed_h[:].bitcast(U16).rearrange(
                "p (t e) -> p t e", e=D // 2)
            for kk in range(KK):
                scr = dram_pool.tile([HG, P], U16, tag=f"scr{kk}",
                                     name=f"scr_{kk}_{h}")
                nc.sync.dma_start(
                    out=scr[:].rearrange("(p t) c -> p t c", p=P),
                    in_=nview[:, :, kk * P:(kk + 1) * P])
                nc.sync.dma_start_transpose(
                    rT[g][kk][:, hh * HG:(hh + 1) * HG], scr[:])
                if g == 0:
                    # unpack own columns for contiguous lhsT
                    src8 = rT[g][kk][:, hh * HG:(hh + 1) * HG].bitcast(
                        FP8).rearrange("p (c i) -> p i c", i=2)
                    dst8 = rT0[kk][:].rearrange(
                        "p (i c) -> p i c", i=2)[:, :, hh * HG:(hh + 1) * HG]
                    nc.vector.tensor_copy(dst8, src8)

        def cs_mm(cs, m, et):
            for ns in range(NS):
                nc.tensor.matmul(
                    cs[ns][:, :],
                    lhsT=ones_bf[:],
                    rhs=et[:, ns * 512:(ns + 1) * 512],
                    start=(m == 0), stop=(m == MB - 1))

        _junk_n = [0]

        def extract(dst_col, src_tile, m):
            _junk_n[0] += 1
            junk = junk_pool.tile([P, P], FP32, tag="junk",
                                  name=f"junk_{_junk_n[0]}")
            nc.vector.scalar_tensor_tensor(
                out=junk[:], in0=src_tile[:, m * P:(m + 1) * P],
                scalar=1.0, in1=ident[:],
                op0=ALU.mult, op1=ALU.mult, accum_out=dst_col)

        def phase_b(nb):
            do_cs = 1 <= nb <= NCS
            cs = None
            pend = []  # lagged ops: (kind, m, tile)
            if do_cs:
                cs = [cs_pool.tile([1, 512], FP32, tag="cs",
                                   name=f"cs_{nb}_{ns}")
                      for ns in range(NS)]
            for m in range(MB):
                ps = psum_pool.tile([P, GB], FP32, tag="ps",
                                    name=f"ps_{nb}_{m}")
                for kk in range(KK):
                    lhsT = rT0[kk][:].rearrange(
                        "p (i c) -> p i c", i=KK)[:, :, m * P:(m + 1) * P]
                    rview = rT[nb][kk][:].bitcast(FP8).rearrange(
                        "p (q i) -> p i q", i=2)
                    for ns in range(NS):
                        nc.tensor.matmul(
                            ps[:, ns * 512:(ns + 1) * 512],
                            lhsT=lhsT,
                            rhs=rview[:, :, ns * 512:(ns + 1) * 512],
                            start=(kk == 0), stop=(kk == KK - 1),
                            perf_mode=DR)
                et = exp_pool.tile([P, GB], BF16, tag="et",
                                   name=f"et_{nb}_{m}")
                nc.scalar.activation(
                    et[:], ps[:], AF.Exp, scale=T_INV / SS,
                    accum_out=rs_all[:, m * NBLK + nb:m * NBLK + nb + 1])
                # lag dependent work one m step so the PE/DVE never stall
                # on the ACT exp of the current tile
                for kind, lm, lt in pend:
                    if kind == "cs":
                        cs_mm(cs, lm, lt)
                    elif kind == "self":
                        extract(e_self[:, lm:lm + 1], lt, lm)
                    else:
                        extract(pos[:, lm:lm + 1], lt, lm)
                pend = []
                if do_cs:
                    pend.append(("cs", m, et[:]))
                if nb == 0:
                    pend.append(("self", m, et[:]))
                if nb == 4:
                    pend.append(("pos", m, ps[:]))
            for kind, lm, lt in pend:
                if kind == "cs":
                    cs_mm(cs, lm, lt)
                elif kind == "self":
                    extract(e_self[:, lm:lm + 1], lt, lm)
                else:
                    extract(pos[:, lm:lm + 1], lt, lm)
            if do_cs:
                # stage into csums permuted: cs[ns][0, q] (q = m*128 + j)
                # -> csums[0, (nb-1)*1024 + 8j + m]
                base = (nb - 1) * GB
                mpc = 512 // P  # m-blocks per cs chunk
                for ns in range(NS):
                    dst = csums[:, base:base + GB].rearrange(
                        "o (j m) -> o m j",
                        m=MB)[:, mpc * ns:mpc * (ns + 1), :]
                    src = cs[ns][:, :].rearrange("o (m j) -> o m j", j=P)
                    nc.vector.tensor_copy(dst, src)

        # emission order: A half-groups run one block ahead of B; blocks
        # 1,2,3 first so the AllGather kicks early, then 4 and 0 hide it.
        phase_a_half(0)
        phase_a_half(1)  # group 0 done (lhsT ready)
        phase_a_half(2)
        phase_a_half(3)  # group 1 done
        for bi, nb in enumerate(BLOCK_ORDER):
            nxt = 2 * (bi + 2)
            if nxt < n_halves:
                phase_a_half(nxt)
                phase_a_half(nxt + 1)
            phase_b(nb)
            if nb == NCS:
                nc.sync.dma_start(out=snd[:], in_=csums[:])
                nc.gpsimd.collective_compute(
                    "AllGather", ALU.bypass,
                    replica_groups=[list(range(N_CORES))],
                    ins=[snd[:].opt()], outs=[gat[:].opt()])

        # pull the gathered [8, 3072] back; slot s = src*3 + (d-1); the
        # sender already permuted so position 8j + m = our row m*128 + j.
        gat_sb = epi_pool.tile([P, N_CORES * NCS * MB], FP32, tag="gat",
                               name="gat_sb")
        nc.sync.dma_start(
            out=gat_sb[:].rearrange("j (r c m) -> j r c m", r=N_CORES,
                                    c=NCS),
            in_=gat[:].rearrange("r (c j m) -> j r c m", c=NCS, j=P))

        # extra[j, m] = sum_s cmask[s] * gat_sb[j, s, m]
        masked = epi_pool.tile([P, N_CORES * NCS * MB], FP32, tag="msk",
                               name="masked")
        nc.vector.tensor_mul(masked[:], gat_sb[:], cmask_sb[:])
        extra = epi_pool.tile([P, MB], FP32, tag="extra", name="extra")
        nc.vector.tensor_reduce(
            extra[:], masked[:].rearrange("j (s m) -> j m s", m=MB),
            axis=AX.X, op=ALU.add)

        # --- epilogue ------------------------------------------------------
        sums = epi_pool.tile([P, MB], FP32, tag="sums", name="sums")
        nc.vector.tensor_reduce(
            sums[:], rs_all[:].rearrange("p (m b) -> p m b", b=NBLK),
            axis=AX.X, op=ALU.add)
        sums2 = epi_pool.tile([P, MB], FP32, tag="sums2", name="sums2")
        nc.vector.tensor_add(sums2[:], sums[:], extra[:])
        denom = epi_pool.tile([P, MB], FP32, tag="denom", name="denom")
        nc.vector.tensor_sub(denom[:], sums2[:], e_self[:])
        ld = epi_pool.tile([P, MB], FP32, tag="ld", name="ld")
        nc.scalar.activation(ld[:], denom[:], AF.Ln)
        # partial = ld - (2/SS)*pos   (pos holds 256*sim)
        part = epi_pool.tile([P, MB], FP32, tag="part", name="part")
        nc.vector.scalar_tensor_tensor(
            out=part[:], in0=pos[:], scalar=-T_INV / SS, in1=ld[:],
            op0=ALU.mult, op1=ALU.add)
        rowtot = epi_pool.tile([P, 1], FP32, tag="rowtot", name="rowtot")
        nc.vector.tensor_reduce(rowtot[:], part[:], axis=AX.X, op=ALU.add)
        pfin = psum_pool.tile([P, GB], FP32, tag="ps", name="pfin")
        nc.tensor.matmul(pfin[:1, :1], lhsT=ones[:], rhs=rowtot[:])
        out_sb = epi_pool.tile([1, 1], FP32, tag="osb", name="out_sb")
        nc.vector.tensor_copy(out_sb[:], pfin[:1, :1])
        nc.sync.dma_start(out=out[:, :], in_=out_sb[:])

    with mock.patch("concourse.bacc.get_activation_tables",
                    _filtered_activation_tables):
        nc.compile()
    return nc


_CACHE_LOCK = threading.Lock()
_CACHED_NC = None


def _get_nc():
    global _CACHED_NC
    with _CACHE_LOCK:
        if _CACHED_NC is None:
            _CACHED_NC = _build_kernel()
        return _CACHED_NC


def _make_cmask(c):
    """0/1 pick of the 3 gathered colsum chunks core c needs: slot
    s = src*3 + (d-1) is wanted iff src == (c - d) mod 8."""
    m = np.zeros(N_CORES * NCS, np.float32)
    for d in range(1, NCS + 1):
        src = (c - d) % N_CORES
        m[src * NCS + (d - 1)] = 1.0
    m = np.repeat(m, MB)  # [24*8], s-major then m
    return np.broadcast_to(m, (P, m.size)).copy()


def _run(inputs, trace=False):
    z_i = np.asarray(inputs["z_i"], dtype=np.float32)
    z_j = np.asarray(inputs["z_j"], dtype=np.float32)
    reps = np.concatenate([z_i, z_j], axis=0)
    in_maps = []
    for c in range(N_CORES):
        rolled = np.roll(reps, -ROWS_PER_CORE * c, axis=0)[:NROWS]
        in_maps.append({
            "reps": np.ascontiguousarray(rolled),
            "cmask": _make_cmask(c),
        })
    nc = _get_nc()
    res = run_bass_kernel_spmd(nc, in_maps, list(range(N_CORES)), trace=trace)
    partials = [float(res.results[i]["out"][0, 0]) for i in range(N_CORES)]
    loss = np.float32(np.sum(np.asarray(partials, dtype=np.float64)) / TWO_N)
    return loss, res


def kernel(**inputs):
    loss, _ = _run(inputs, trace=False)
    return np.asarray(loss, dtype=np.float32)
